# revision 1
# baseline (speedup 1.0000x reference)
"""CrystalGCN (3x CGConv + mean-pool + linear) Trainium2 Bass kernel, 8-core SPMD.

Strategy:
  - Host: relabel nodes so each core owns 4096 node slots (32 windows x 128),
    graphs are assigned whole to cores (pooling stays core-local), and window
    in-degrees are balanced (bin-packing) so every window has <= T_W*128 edges.
  - Device per layer: h lives as a replicated fp16 DRAM table [32768,128].
    Per 128-edge tile: transpose-gather h[dst], h[src] as fp16 [128f,128e]
    (direct matmul lhsT), 3 PSUM-accumulated matmuls (dst, src, edge_attr+bias)
    -> pre[128e, 256] = [f-side | s-side]; sigmoid via 1/(1+exp(-a)), softplus
    via ln(1+exp(b)) (one ACT table set); msg = sig*sp; scatter-add via
    selection-matrix matmul into per-window PSUM. Window flush: relu(h+acc).
    AllGather fp16 h shards between layers.
  - Pooling: per-node scale by 1/cnt(graph), selection-matmul into [graph,128],
    transpose, project by Wlin, add blin; host assembles [1600,128].
"""
import numpy as np
import ml_dtypes

import concourse.bacc as bacc
import concourse.mybir as mybir
import concourse.tile as tile
from concourse import library_config
from concourse.bass_utils import run_bass_kernel_spmd

FP32 = mybir.dt.float32
FP16 = mybir.dt.float16
I16 = mybir.dt.int16
AF = mybir.ActivationFunctionType
OP = mybir.AluOpType

N_CORES = 8
N_NODES = 32000
N_EDGES = 320000
N_GRAPHS = 1600
HID = 128
RBF = 32
NODES_PC = 4096          # node slots per core
WINDOWS_PC = 32          # windows per core (128 nodes each)
GRAPHS_PC = 256          # graph slots per core (2 windows of 128)
V_PAD = N_CORES * NODES_PC  # 32768 total node slots (int16-safe)

_f16 = ml_dtypes.float16 if hasattr(ml_dtypes, "float16") else np.float16


# ---------------------------------------------------------------- host prep --
def _wrap_idxs(idx: np.ndarray) -> np.ndarray:
    """[n] -> [128, n/16] int16 (16-partition wrap, replicated x8 gpsimd cores)."""
    n = idx.shape[0]
    assert n % 16 == 0
    w = idx.astype(np.int16).reshape(n // 16, 16).T
    return np.ascontiguousarray(np.tile(w, (8, 1)))


def _bin_pack(sizes, n_bins, cap_items, cap_extra=None, extra=None):
    """Greedy: big first into least-loaded bin with room. Returns bin id/array."""
    order = np.argsort(-sizes, kind="stable")
    load = np.zeros(n_bins, dtype=np.int64)
    items = np.zeros(n_bins, dtype=np.int64)
    ext = np.zeros(n_bins, dtype=np.int64)
    out = np.zeros(len(sizes), dtype=np.int64)
    for i in order:
        ok = items < cap_items
        if cap_extra is not None:
            ok &= (ext + extra[i]) <= cap_extra
        cand = np.where(ok)[0]
        b = cand[np.argmin(load[cand])]
        out[i] = b
        load[b] += sizes[i]
        items[b] += 1
        if cap_extra is not None:
            ext[b] += extra[i]
    return out


def preprocess(x, edge_index, edge_attr, batch):
    """Build all per-core device arrays. Returns dict of host data."""
    x = np.asarray(x).astype(np.int64)
    src = np.asarray(edge_index[0]).astype(np.int64)
    dst = np.asarray(edge_index[1]).astype(np.int64)
    ea = np.asarray(edge_attr).astype(np.float32)
    batch = np.asarray(batch).astype(np.int64)

    deg = np.bincount(dst, minlength=N_NODES)
    g_nodes = np.bincount(batch, minlength=N_GRAPHS)
    g_edges = np.zeros(N_GRAPHS, dtype=np.int64)
    np.add.at(g_edges, batch, deg)

    # graphs -> cores (balance edges; cap nodes/graphs per core)
    g_core = _bin_pack(g_edges, N_CORES, GRAPHS_PC, NODES_PC, g_nodes)
    # local graph slot per graph
    g_slot = np.zeros(N_GRAPHS, dtype=np.int64)
    for c in range(N_CORES):
        ids = np.where(g_core == c)[0]
        g_slot[ids] = np.arange(len(ids))

    # nodes -> windows within core (balance in-degree; cap 128 nodes/window)
    node_core = g_core[batch]
    new_id = np.zeros(N_NODES, dtype=np.int64)
    for c in range(N_CORES):
        ids = np.where(node_core == c)[0]
        w = _bin_pack(deg[ids].astype(np.int64), WINDOWS_PC, 128)
        slot = np.zeros(len(ids), dtype=np.int64)
        for wi in range(WINDOWS_PC):
            m = np.where(w == wi)[0]
            slot[m] = np.arange(len(m))
        new_id[ids] = c * NODES_PC + w * 128 + slot

    # edges keyed by destination window
    nd = new_id[dst]
    ns = new_id[src]
    wkey = nd >> 7  # global window id 0..255
    order = np.argsort(wkey, kind="stable")
    nd, ns, wkey = nd[order], ns[order], wkey[order]
    ea_s = ea[order]
    wcnt = np.bincount(wkey, minlength=N_CORES * WINDOWS_PC)
    t_w = int(np.ceil(wcnt.max() / 128.0))
    t_w += t_w % 2  # even
    epw = t_w * 128                    # padded edges per window
    e_pad = WINDOWS_PC * epw           # padded edges per core

    # slot position for each (sorted) edge: window_start_pad + rank_in_window
    starts = np.zeros(N_CORES * WINDOWS_PC + 1, dtype=np.int64)
    np.cumsum(wcnt, out=starts[1:])
    rank = np.arange(len(nd)) - starts[wkey]
    pos = wkey * epw + rank            # global padded position

    srcw = np.zeros(N_CORES * e_pad, dtype=np.int64)
    dstw = np.zeros(N_CORES * e_pad, dtype=np.int64)
    ldw = np.full(N_CORES * e_pad, 255.0, dtype=np.float32)
    eaw = np.zeros((N_CORES * e_pad, RBF), dtype=np.float32)
    srcw[pos] = ns
    dstw[pos] = nd
    ldw[pos] = (nd & 127).astype(np.float32)
    eaw[pos] = ea_s

    # per-node pooling metadata (by new node id)
    inv_cnt = np.zeros(V_PAD, dtype=np.float32)
    lg = np.full(V_PAD, 512.0, dtype=np.float32)
    cnt = np.maximum(g_nodes, 1).astype(np.float32)
    inv_cnt[new_id] = 1.0 / cnt[batch]
    lg[new_id] = g_slot[batch].astype(np.float32)

    # embedding index per new node id (dummies -> 0)
    embi = np.zeros(V_PAD, dtype=np.int64)
    embi[new_id] = x

    pc = []
    for c in range(N_CORES):
        sl = slice(c * e_pad, (c + 1) * e_pad)
        nsl = slice(c * NODES_PC, (c + 1) * NODES_PC)
        pc.append(dict(
            src_idx=_wrap_idxs(srcw[sl]),
            dst_idx=_wrap_idxs(dstw[sl]),
            ld=np.ascontiguousarray(
                ldw[sl].reshape(-1, 128).T.astype(_f16)),          # [128, tiles]
            eaT=np.ascontiguousarray(
                np.concatenate([eaw[sl].T.astype(np.float32),
                                np.ones((1, e_pad), np.float32)], 0)
                .astype(_f16)),                                     # [33, e_pad]
            emb_own_idx=_wrap_idxs(embi[nsl]),
            inv_cnt=np.ascontiguousarray(
                inv_cnt[nsl].reshape(-1, 128).T.astype(np.float32)),  # [128,32]
            lg0=np.ascontiguousarray(
                lg[nsl].reshape(-1, 128).T.astype(_f16)),             # [128,32]
            lg1=np.ascontiguousarray(
                (lg[nsl].reshape(-1, 128).T - 128.0).astype(_f16)),
        ))
    return dict(per_core=pc, t_w=t_w, e_pad=e_pad,
                emb_idx=_wrap_idxs(embi), g_core=g_core, g_slot=g_slot)


# ---------------------------------------------------------------- device ----
def build_program(t_w: int, e_pad: int):
    nc = bacc.Bacc("TRN2", target_bir_lowering=False, debug=False,
                   enable_asserts=False, num_devices=N_CORES)
    n_tiles = e_pad // 128
    GW = 4                       # windows per gather group
    GN = GW * t_w * 128          # idxs per gather
    n_grp = WINDOWS_PC // GW

    def din(name, shape, dt):
        return nc.dram_tensor(name, shape, dt, kind="ExternalInput").ap()

    emb16 = din("emb16", [128, HID], FP16)
    emb32 = din("emb32", [128, HID], FP32)
    src_idx = din("src_idx", [128, e_pad // 16], I16)
    dst_idx = din("dst_idx", [128, e_pad // 16], I16)
    ld_d = din("ld", [128, n_tiles], FP16)
    eaT_d = din("eaT", [33, e_pad], FP16)
    emb_idx = din("emb_idx", [128, V_PAD // 16], I16)
    emb_own = din("emb_own_idx", [128, NODES_PC // 16], I16)
    invc_d = din("inv_cnt", [128, WINDOWS_PC], FP32)
    lg0_d = din("lg0", [128, WINDOWS_PC], FP16)
    lg1_d = din("lg1", [128, WINDOWS_PC], FP16)
    iota_d = din("iota", [128, 128], FP16)
    ident_d = din("ident", [128, 128], FP16)
    wdst_d = din("wdst", [3, 128, 2 * HID], FP16)
    wsrc_d = din("wsrc", [3, 128, 2 * HID], FP16)
    wea_d = din("wea", [3, 33, 2 * HID], FP16)
    wlin_d = din("wlin", [128, 128], FP16)
    blin_d = din("blin", [128, 1], FP32)
    out_ext = nc.dram_tensor("outT", [128, GRAPHS_PC], FP32,
                             kind="ExternalOutput").ap()

    with tile.TileContext(nc) as tc:
        with (
            tc.tile_pool(name="const", bufs=1) as cpool,
            tc.tile_pool(name="persist", bufs=1) as ppool,
            tc.tile_pool(name="gath", bufs=2) as gpool,
            tc.tile_pool(name="work", bufs=3) as wpool,
            tc.tile_pool(name="dram", bufs=1, space="DRAM") as dr,
        ):
            nc.gpsimd.load_library(library_config.mlp)

            # ---- constants to SBUF
            iota_sb = cpool.tile([128, 128], FP16)
            nc.sync.dma_start(out=iota_sb[:], in_=iota_d)
            ident_sb = cpool.tile([128, 128], FP16)
            nc.sync.dma_start(out=ident_sb[:], in_=ident_d)
            ld_sb = cpool.tile([128, n_tiles], FP16)
            nc.sync.dma_start(out=ld_sb[:], in_=ld_d)
            srci_sb = cpool.tile([128, e_pad // 16], I16)
            nc.sync.dma_start(out=srci_sb[:], in_=src_idx)
            dsti_sb = cpool.tile([128, e_pad // 16], I16)
            nc.sync.dma_start(out=dsti_sb[:], in_=dst_idx)
            embi_sb = cpool.tile([128, V_PAD // 16], I16)
            nc.sync.dma_start(out=embi_sb[:], in_=emb_idx)
            embo_sb = cpool.tile([128, NODES_PC // 16], I16)
            nc.sync.dma_start(out=embo_sb[:], in_=emb_own)
            invc_sb = cpool.tile([128, WINDOWS_PC], FP32)
            nc.sync.dma_start(out=invc_sb[:], in_=invc_d)
            lg0_sb = cpool.tile([128, WINDOWS_PC], FP16)
            nc.sync.dma_start(out=lg0_sb[:], in_=lg0_d)
            lg1_sb = cpool.tile([128, WINDOWS_PC], FP16)
            nc.sync.dma_start(out=lg1_sb[:], in_=lg1_d)
            wdst_sb = cpool.tile([128, 3 * 2 * HID], FP16)
            nc.sync.dma_start(
                out=wdst_sb[:].rearrange("p (l n) -> p l n", l=3),
                in_=wdst_d.rearrange("l p n -> p l n"))
            wsrc_sb = cpool.tile([128, 3 * 2 * HID], FP16)
            nc.sync.dma_start(
                out=wsrc_sb[:].rearrange("p (l n) -> p l n", l=3),
                in_=wsrc_d.rearrange("l p n -> p l n"))
            wea_sb = cpool.tile([33, 3 * 2 * HID], FP16)
            nc.sync.dma_start(
                out=wea_sb[:].rearrange("p (l n) -> p l n", l=3),
                in_=wea_d.rearrange("l p n -> p l n"))
            wlin_sb = cpool.tile([128, 128], FP16)
            nc.sync.dma_start(out=wlin_sb[:], in_=wlin_d)
            blin_sb = cpool.tile([128, 1], FP32)
            nc.sync.dma_start(out=blin_sb[:], in_=blin_d)
            emb16_sb = cpool.tile([128, HID], FP16)
            nc.sync.dma_start(out=emb16_sb[:], in_=emb16)

            # h tables in DRAM (fp16), one per layer input
            tabs = [dr.tile([V_PAD, HID], FP16, tag=f"tab{i}",
                            name=f"tab{i}",
                            addr_space="Shared" if i else "Local")
                    for i in range(3)]
            ag_in = dr.tile([NODES_PC, HID], FP16, tag="ag_in")

            # persistent fp32 own-h  [p, w, f] ; slot p of window w
            h_own = ppool.tile([128, WINDOWS_PC, HID], FP32)
            hn16 = ppool.tile([128, WINDOWS_PC, HID], FP16)

            # ---- phase 0: build h0 table (fp16) + own h (fp32)
            for ch in range(8):
                st = wpool.tile([128, 32, HID], FP16, tag="h0st")
                nc.gpsimd.dma_gather(
                    st[:], emb16, embi_sb[:, ch * 256:(ch + 1) * 256],
                    4096, 4096, elem_size=HID, transpose=False, single_packet=False)
                nc.sync.dma_start(
                    out=tabs[0][ch * 4096:(ch + 1) * 4096, :]
                        .rearrange("(t p) f -> p t f", p=128),
                    in_=st[:])
            nc.gpsimd.dma_gather(
                h_own[:], emb32, embo_sb[:], NODES_PC, NODES_PC,
                elem_size=HID, transpose=False, single_packet=False)

            # ---- layers
            with tc.tile_pool(name="psum_e", bufs=1, space="PSUM") as pse:
                for l in range(3):
                    tab = tabs[l]
                    for grp in range(n_grp):
                        c0 = grp * GN
                        hdT = gpool.tile([128, 1, GN], FP16, tag="hdT")
                        nc.gpsimd.dma_gather(
                            hdT[:], tab[:], dsti_sb[:, c0 // 16:(c0 + GN) // 16],
                            GN, GN, elem_size=HID, transpose=True, single_packet=False)
                        hsT = gpool.tile([128, 1, GN], FP16, tag="hsT")
                        nc.gpsimd.dma_gather(
                            hsT[:], tab[:], srci_sb[:, c0 // 16:(c0 + GN) // 16],
                            GN, GN, elem_size=HID, transpose=True, single_packet=False)
                        eag = gpool.tile([33, GN], FP16, tag="eag")
                        nc.sync.dma_start(out=eag[:], in_=eaT_d[:, c0:c0 + GN])

                        for wi in range(GW):
                            w = grp * GW + wi
                            acc = pse.tile([128, HID], FP32, tag="acc", bufs=2)
                            for pr in range(t_w // 2):
                                pre = pse.tile([128, 512], FP32, tag="pre", bufs=3)
                                S = wpool.tile([128, 256], FP16, tag="S")
                                for hf in range(2):
                                    ti = pr * 2 + hf
                                    e0 = wi * t_w * 128 + ti * 128
                                    te = w * t_w + ti
                                    po = pre[:, hf * 256:(hf + 1) * 256]
                                    nc.tensor.matmul(
                                        po, lhsT=hdT[:, 0, e0:e0 + 128],
                                        rhs=wdst_sb[:, l * 256:(l + 1) * 256],
                                        start=True, stop=False)
                                    nc.tensor.matmul(
                                        po, lhsT=hsT[:, 0, e0:e0 + 128],
                                        rhs=wsrc_sb[:, l * 256:(l + 1) * 256],
                                        start=False, stop=False)
                                    nc.tensor.matmul(
                                        po, lhsT=eag[:, e0:e0 + 128],
                                        rhs=wea_sb[:, l * 256:(l + 1) * 256],
                                        start=False, stop=True)
                                    nc.vector.tensor_tensor(
                                        out=S[:, hf * 128:(hf + 1) * 128],
                                        in0=ld_sb[:, te:te + 1]
                                            .to_broadcast([128, 128]),
                                        in1=iota_sb[:], op=OP.is_equal)
                                pre3 = pre[:].rearrange("p (t h) -> p t h", h=256)
                                fb = wpool.tile([128, 256], FP32, tag="fb")
                                nc.scalar.activation(fb[:], pre3[:, :, 0:128],
                                                     AF.Exp, scale=-1.0)
                                sb2 = wpool.tile([128, 256], FP32, tag="sb2")
                                nc.scalar.activation(sb2[:], pre3[:, :, 128:256],
                                                     AF.Exp)
                                nc.vector.tensor_scalar_add(fb[:], fb[:], 1.0)
                                nc.vector.tensor_scalar_add(sb2[:], sb2[:], 1.0)
                                nc.vector.reciprocal(fb[:], fb[:])
                                nc.scalar.activation(sb2[:], sb2[:], AF.Ln)
                                msg = wpool.tile([128, 256], FP16, tag="msg")
                                nc.vector.tensor_mul(msg[:], fb[:], sb2[:])
                                for hf in range(2):
                                    nc.tensor.matmul(
                                        acc[:],
                                        lhsT=S[:, hf * 128:(hf + 1) * 128],
                                        rhs=msg[:, hf * 128:(hf + 1) * 128],
                                        start=(pr == 0 and hf == 0),
                                        stop=(pr == t_w // 2 - 1 and hf == 1))
                            # window flush: h = relu(h + acc)
                            hn = wpool.tile([128, HID], FP32, tag="hn")
                            nc.vector.tensor_add(hn[:], acc[:], h_own[:, w, :])
                            nc.vector.tensor_scalar_max(hn[:], hn[:], 0.0)
                            nc.vector.tensor_copy(h_own[:, w, :], hn[:])
                            if l < 2:
                                nc.vector.tensor_copy(hn16[:, w, :], hn[:])
                    if l < 2:
                        nc.sync.dma_start(
                            out=ag_in[:].rearrange("(w p) f -> p w f", p=128),
                            in_=hn16[:])
                        nc.gpsimd.collective_compute(
                            "AllGather", OP.bypass,
                            replica_groups=[list(range(N_CORES))],
                            ins=[ag_in[:]], outs=[tabs[l + 1][:]])

            # ---- pooling + final linear
            with tc.tile_pool(name="psum_p", bufs=1, space="PSUM") as psp:
                pa0 = psp.tile([128, HID], FP32, tag="pa0")
                pa1 = psp.tile([128, HID], FP32, tag="pa1")
                for t in range(WINDOWS_PC):
                    sc = wpool.tile([128, HID], FP16, tag="sc")
                    nc.vector.tensor_mul(
                        sc[:], h_own[:, t, :],
                        invc_sb[:, t:t + 1].to_broadcast([128, HID]))
                    sg = wpool.tile([128, 256], FP16, tag="sg")
                    nc.vector.tensor_tensor(
                        out=sg[:, 0:128],
                        in0=lg0_sb[:, t:t + 1].to_broadcast([128, 128]),
                        in1=iota_sb[:], op=OP.is_equal)
                    nc.vector.tensor_tensor(
                        out=sg[:, 128:256],
                        in0=lg1_sb[:, t:t + 1].to_broadcast([128, 128]),
                        in1=iota_sb[:], op=OP.is_equal)
                    nc.tensor.matmul(pa0[:], lhsT=sg[:, 0:128], rhs=sc[:],
                                     start=(t == 0), stop=(t == WINDOWS_PC - 1))
                    nc.tensor.matmul(pa1[:], lhsT=sg[:, 128:256], rhs=sc[:],
                                     start=(t == 0), stop=(t == WINDOWS_PC - 1))
                pooledT = wpool.tile([128, 256], FP16, tag="pooledT")
                for i, pa in enumerate((pa0, pa1)):
                    pc16 = wpool.tile([128, 128], FP16, tag="pc16")
                    nc.vector.tensor_copy(pc16[:], pa[:])
                    pt = psp.tile([128, 128], FP16, tag="pt")
                    nc.tensor.transpose(out=pt[:], in_=pc16[:],
                                        identity=ident_sb[:])
                    nc.vector.tensor_copy(pooledT[:, i * 128:(i + 1) * 128],
                                          pt[:])
                op_ps = psp.tile([128, GRAPHS_PC], FP32, tag="op")
                nc.tensor.matmul(op_ps[:], lhsT=wlin_sb[:], rhs=pooledT[:],
                                 start=True, stop=True)
                outs = wpool.tile([128, GRAPHS_PC], FP32, tag="outs")
                nc.scalar.activation(outs[:], op_ps[:], AF.Identity,
                                     bias=blin_sb[:, 0:1])
                nc.sync.dma_start(out=out_ext, in_=outs[:])
    nc.finalize()
    return nc


# ---------------------------------------------------------------- kernel ----
_CACHE = {}


def kernel(x, edge_index, edge_attr, batch, emb,
           Wf1, bf1, Ws1, bs1, Wf2, bf2, Ws2, bs2, Wf3, bf3, Ws3, bs3,
           Wlin, blin, _return_extras=False):
    prep = preprocess(x, edge_index, edge_attr, batch)
    t_w, e_pad = prep["t_w"], prep["e_pad"]

    key = (t_w, e_pad)
    if key not in _CACHE:
        _CACHE[key] = build_program(t_w, e_pad)
    nc = _CACHE[key]

    emb = np.asarray(emb, np.float32)
    emb_pad = np.zeros((128, HID), np.float32)
    emb_pad[:emb.shape[0]] = emb
    Wf = [np.asarray(w, np.float32) for w in (Wf1, Wf2, Wf3)]
    Ws = [np.asarray(w, np.float32) for w in (Ws1, Ws2, Ws3)]
    bf = [np.asarray(b, np.float32) for b in (bf1, bf2, bf3)]
    bs = [np.asarray(b, np.float32) for b in (bs1, bs2, bs3)]
    wdst = np.stack([np.concatenate([Wf[i][0:128], Ws[i][0:128]], 1)
                     for i in range(3)]).astype(_f16)          # [3,128,256]
    wsrc = np.stack([np.concatenate([Wf[i][128:256], Ws[i][128:256]], 1)
                     for i in range(3)]).astype(_f16)
    wea = np.stack([np.concatenate(
        [np.concatenate([Wf[i][256:288], Ws[i][256:288]], 1),
         np.concatenate([bf[i], bs[i]])[None, :]], 0)
        for i in range(3)]).astype(_f16)                       # [3,33,256]
    iota = np.tile(np.arange(128, dtype=np.float32)[None, :],
                   (128, 1)).astype(_f16)
    ident = np.eye(128, dtype=np.float32).astype(_f16)

    common = dict(
        emb16=emb_pad.astype(_f16), emb32=emb_pad,
        emb_idx=prep["emb_idx"], iota=iota, ident=ident,
        wdst=wdst, wsrc=wsrc, wea=wea,
        wlin=np.ascontiguousarray(np.asarray(Wlin, np.float32)).astype(_f16),
        blin=np.asarray(blin, np.float32).reshape(128, 1),
    )
    in_maps = [{**common, **{k: v for k, v in prep["per_core"][c].items()
                             if k != "emb_own_idx"},
                "emb_own_idx": prep["per_core"][c]["emb_own_idx"]}
               for c in range(N_CORES)]

    res = run_bass_kernel_spmd(nc, in_maps, core_ids=list(range(N_CORES)),
                               trace=False)
    outT = [res.results[c]["outT"] for c in range(N_CORES)]  # [128, 256] each
    g_core, g_slot = prep["g_core"], prep["g_slot"]
    out = np.zeros((N_GRAPHS, HID), np.float32)
    for g in range(N_GRAPHS):
        out[g] = outT[g_core[g]][:, g_slot[g]]
    if _return_extras:
        return out, res
    return out

